# revision 1
# baseline (speedup 1.0000x reference)
"""AgentCollisionLoss Trainium2 kernel.

Sharding: 8 cores = B(4) x i-half(2). Core c handles b = c//2 and the 32
i-agents [h*32, h*32+32) vs all 64 j-agents over all T=80 steps.

On-device per core:
  world disk coords wx/wy for all 64 agents  (partitions = j)
  loop over 16 i-pairs: partitions p = (i-parity)*64 + j, free = (t,k,l)
    dx = xi - xj, dy = yi - yj           (DVE tensor_tensor, fp32)
    sqx = dx^2, sqy = dy^2               (ACT Square)
    d2 = sqx + sqy                       (GPSIMD scalar_tensor_tensor)
    dmin2 = min over (k,l)               (DVE tensor_reduce)
  clamp, sqrt (ACT), q = dmin/pd, pen = min(q-1, 0), weight+sum
Host: slice inputs per core, sum the 8x[128] partials, negate.
"""

import numpy as np

import concourse.bass as bass
import concourse.bacc as bacc
import concourse.tile as tile
import concourse.mybir as mybir
from concourse import bass_utils

B, N, T, D = 4, 64, 80, 6
K = 5
NCORES = 8
BUFFER_DIST = 0.2
DECAY_RATE = 0.9
TK = T * K          # 400
TKL = T * K * K     # 2000
NPAIR = 16          # i-pairs per core (32 i-agents / 2)
FD = mybir.dt.float32

_CACHE = {}
_LAST_INMAPS = None


def _build():
    nc = bacc.Bacc("TRN2", target_bir_lowering=False, debug=False,
                   num_devices=NCORES)

    y_in = nc.dram_tensor("y_in", [N, T * D], FD, kind="ExternalInput").ap()
    len_in = nc.dram_tensor("len_in", [N, 1], FD, kind="ExternalInput").ap()
    wid_in = nc.dram_tensor("wid_in", [N, 1], FD, kind="ExternalInput").ap()
    frac_in = nc.dram_tensor("frac_in", [K], FD, kind="ExternalInput").ap()
    wt_in = nc.dram_tensor("wt_in", [128, NPAIR * T], FD,
                           kind="ExternalInput").ap()
    part_out = nc.dram_tensor("part_out", [128, 1], FD,
                              kind="ExternalOutput").ap()

    stage_x = nc.dram_tensor("stage_x", [N * TK], FD, kind="Internal").ap()
    stage_y = nc.dram_tensor("stage_y", [N * TK], FD, kind="Internal").ap()
    stage_r = nc.dram_tensor("stage_r", [N], FD, kind="Internal").ap()

    # the per-core i-half offset is baked per-core via h below; we build one
    # program (SPMD) so instead we bake h into the *host-provided* Wt and by
    # providing per-core inputs; the i-slice offset must be identical across
    # cores, so the host rotates the agent axis per core such that the
    # i-agents are always rows [0, 32).
    HOFF = 0

    with tile.TileContext(nc) as tc:
        with (
            tc.tile_pool(name="prep", bufs=1) as prep,
            tc.tile_pool(name="rep", bufs=1) as rep,
            tc.tile_pool(name="xiyi", bufs=2) as xiyi,
            tc.tile_pool(name="work", bufs=1) as work,
            tc.tile_pool(name="acc", bufs=1) as acc,
        ):
            # ---- stage 1: per-agent prep (partitions = j, 64) ----
            ytile = prep.tile([N, T * D], FD)
            nc.sync.dma_start(out=ytile, in_=y_in)
            def ycol(dcol):
                return bass.AP(tensor=ytile.tensor,
                               offset=ytile.offset + dcol,
                               ap=[ytile.ap[0], [D, T]])
            x_ap = ycol(0)
            ypos_ap = ycol(1)
            yaw_ap = ycol(4)

            ltile = prep.tile([N, 1], FD)
            wtile = prep.tile([N, 1], FD)
            nc.sync.dma_start(out=ltile, in_=len_in)
            nc.sync.dma_start(out=wtile, in_=wid_in)
            fr = prep.tile([N, K], FD)
            nc.sync.dma_start(
                out=fr,
                in_=bass.AP(tensor=frac_in.tensor, offset=0,
                            ap=[[0, N], [1, K]]))

            zero128 = prep.tile([128, 1], FD)
            nc.vector.memset(zero128, 0.0)
            pi2 = prep.tile([N, 1], FD)
            nc.vector.memset(pi2, float(np.pi / 2))
            cosT = prep.tile([N, T], FD)
            sinT = prep.tile([N, T], FD)
            nc.scalar.activation(out=cosT, in_=yaw_ap,
                                 func=mybir.ActivationFunctionType.Sin,
                                 bias=pi2, scale=1.0)
            nc.scalar.activation(out=sinT, in_=yaw_ap,
                                 func=mybir.ActivationFunctionType.Sin,
                                 bias=zero128[:N, :], scale=1.0)

            rad = prep.tile([N, 1], FD)
            nc.vector.tensor_scalar(out=rad, in0=wtile, scalar1=0.5,
                                    scalar2=0.0, op0=mybir.AluOpType.mult,
                                    op1=mybir.AluOpType.add)
            # cmax = l/2 - rad ; cmin = -cmax ; cent = cmin + (cmax-cmin)*frac
            cmax = prep.tile([N, 1], FD)
            nc.vector.scalar_tensor_tensor(out=cmax, in0=ltile, scalar=0.5,
                                           in1=rad,
                                           op0=mybir.AluOpType.mult,
                                           op1=mybir.AluOpType.subtract)
            # cent[j,l] = cmax * f2[l]   (host provides f2 = 2*frac-1)
            cent = prep.tile([N, K], FD)
            nc.vector.tensor_scalar(out=cent, in0=fr, scalar1=cmax,
                                    scalar2=0.0,
                                    op0=mybir.AluOpType.mult,
                                    op1=mybir.AluOpType.add)

            wx = prep.tile([N, TK], FD)
            wy = prep.tile([N, TK], FD)
            # wx = cent[j,l]*cos[j,t] + x[j,t]
            tmp = prep.tile([N, TK], FD)

            def bc_tl(src_t):   # [N,T] -> (t,l) view
                return bass.AP(tensor=src_t.tensor, offset=src_t.offset,
                               ap=[src_t.ap[0], [src_t.ap[-1][0], T], [0, K]])

            def bc_lt(src_l):   # [N,K] -> (t,l) view
                return bass.AP(tensor=src_l.tensor, offset=src_l.offset,
                               ap=[src_l.ap[0], [0, T], [src_l.ap[-1][0], K]])

            wx3 = wx[:, :].rearrange("p (t l) -> p t l", l=K)
            wy3 = wy[:, :].rearrange("p (t l) -> p t l", l=K)
            tmp3 = tmp[:, :].rearrange("p (t l) -> p t l", l=K)

            nc.vector.tensor_tensor(out=tmp3, in0=bc_tl(cosT), in1=bc_lt(cent),
                                    op=mybir.AluOpType.mult)
            nc.vector.tensor_tensor(out=wx3, in0=tmp3, in1=bc_tl(x_ap),
                                    op=mybir.AluOpType.add)
            nc.vector.tensor_tensor(out=tmp3, in0=bc_tl(sinT), in1=bc_lt(cent),
                                    op=mybir.AluOpType.mult)
            nc.vector.tensor_tensor(out=wy3, in0=bc_tl(ypos_ap), in1=tmp3,
                                    op=mybir.AluOpType.subtract)

            # ---- stage 2: bounce to DRAM, replicate ----
            nc.sync.dma_start(
                out=bass.AP(tensor=stage_x.tensor, offset=0,
                            ap=[[TK, N], [1, TK]]),
                in_=wx)
            nc.sync.dma_start(
                out=bass.AP(tensor=stage_y.tensor, offset=0,
                            ap=[[TK, N], [1, TK]]),
                in_=wy)
            nc.sync.dma_start(
                out=bass.AP(tensor=stage_r.tensor, offset=0,
                            ap=[[1, N], [1, 1]]),
                in_=rad)

            xj = rep.tile([128, TK], FD)
            yj = rep.tile([128, TK], FD)
            radj = rep.tile([128, 1], FD)
            radi = rep.tile([128, NPAIR], FD)
            for ih in range(2):
                sl = slice(ih * N, (ih + 1) * N)
                nc.sync.dma_start(
                    out=xj[sl, :],
                    in_=bass.AP(tensor=stage_x.tensor, offset=0,
                                ap=[[TK, N], [1, TK]]))
                nc.sync.dma_start(
                    out=yj[sl, :],
                    in_=bass.AP(tensor=stage_y.tensor, offset=0,
                                ap=[[TK, N], [1, TK]]))
                nc.sync.dma_start(
                    out=radj[sl, :],
                    in_=bass.AP(tensor=stage_r.tensor, offset=0,
                                ap=[[1, N], [0, 1]]))
                nc.sync.dma_start(
                    out=radi[sl, :],
                    in_=bass.AP(tensor=stage_r.tensor, offset=HOFF + ih,
                                ap=[[0, N], [2, NPAIR]]))

            wtt = rep.tile([128, NPAIR * T], FD)
            nc.sync.dma_start(out=wtt, in_=wt_in)

            # pd = radi + radj + BUFFER ; invpd = 1/pd
            pd = rep.tile([128, NPAIR], FD)
            nc.vector.tensor_scalar(out=pd, in0=radi, scalar1=radj,
                                    scalar2=float(BUFFER_DIST),
                                    op0=mybir.AluOpType.add,
                                    op1=mybir.AluOpType.add)
            invpd = rep.tile([128, NPAIR], FD)
            nc.vector.reciprocal(out=invpd, in_=pd)

            dminb = acc.tile([128, NPAIR * T], FD)

            def xj_tkl(src):    # [128,(t,k? no t,l)] -> (t,k,l): k bcast
                return bass.AP(tensor=src.tensor, offset=src.offset,
                               ap=[src.ap[0], [K, T], [0, K], [1, K]])

            # ---- stage 3: main loop, two interleaved streams ----
            for g in range(NPAIR // 2):
                ips = (2 * g, 2 * g + 1)
                sfx = ("a", "b")
                xis, yis, dxs, dys, sqxs, sqys, d2s = [], [], [], [], [], [], []
                for s in range(2):
                    ip = ips[s]
                    xi = xiyi.tile([128, TK], FD, tag="xi" + sfx[s])
                    yi = xiyi.tile([128, TK], FD, tag="yi" + sfx[s])
                    off = (HOFF + 2 * ip) * TK
                    for ih in range(2):
                        sl = slice(ih * N, (ih + 1) * N)
                        nc.sync.dma_start(
                            out=xi[sl, :],
                            in_=bass.AP(tensor=stage_x.tensor,
                                        offset=off + ih * TK,
                                        ap=[[0, N], [1, TK]]))
                        nc.sync.dma_start(
                            out=yi[sl, :],
                            in_=bass.AP(tensor=stage_y.tensor,
                                        offset=off + ih * TK,
                                        ap=[[0, N], [1, TK]]))
                    xis.append(xi)
                    yis.append(yi)

                def xi_tkl(src):  # (t,k) -> (t,k,l): l bcast
                    return bass.AP(tensor=src.tensor, offset=src.offset,
                                   ap=[src.ap[0], [K, T], [1, K], [0, K]])

                for s in range(2):
                    dx = work.tile([128, TKL], FD, tag="dx" + sfx[s])
                    dy = work.tile([128, TKL], FD, tag="dy" + sfx[s])
                    dx3 = dx[:, :].rearrange("p (t k l) -> p t k l", k=K, l=K)
                    dy3 = dy[:, :].rearrange("p (t k l) -> p t k l", k=K, l=K)
                    nc.vector.tensor_tensor(out=dx3, in0=xi_tkl(xis[s]),
                                            in1=xj_tkl(xj),
                                            op=mybir.AluOpType.subtract)
                    nc.vector.tensor_tensor(out=dy3, in0=xi_tkl(yis[s]),
                                            in1=xj_tkl(yj),
                                            op=mybir.AluOpType.subtract)
                    dxs.append(dx)
                    dys.append(dy)
                for s in range(2):
                    sqx = work.tile([128, TKL], FD, tag="sqx" + sfx[s])
                    sqy = work.tile([128, TKL], FD, tag="sqy" + sfx[s])
                    nc.scalar.activation(out=sqx, in_=dxs[s],
                                         func=mybir.ActivationFunctionType.Square,
                                         bias=zero128)
                    nc.scalar.activation(out=sqy, in_=dys[s],
                                         func=mybir.ActivationFunctionType.Square,
                                         bias=zero128)
                    sqxs.append(sqx)
                    sqys.append(sqy)
                for s in range(2):
                    d2 = work.tile([128, TKL], FD, tag="d2" + sfx[s])
                    nc.gpsimd.tensor_tensor(out=d2, in0=sqxs[s], in1=sqys[s],
                                            op=mybir.AluOpType.add)
                    d2s.append(d2)
                for s in range(2):
                    ip = ips[s]
                    d23 = d2s[s][:, :].rearrange("p (t kl) -> p t kl",
                                                 kl=K * K)
                    nc.vector.tensor_reduce(
                        out=dminb[:, ip * T:(ip + 1) * T], in_=d23,
                        axis=mybir.AxisListType.X, op=mybir.AluOpType.min)

            # ---- stage 4: finish ----
            nc.vector.tensor_scalar(out=dminb, in0=dminb, scalar1=0.0,
                                    scalar2=0.0, op0=mybir.AluOpType.max,
                                    op1=mybir.AluOpType.add)
            dist = acc.tile([128, NPAIR * T], FD)
            nc.scalar.activation(out=dist, in_=dminb,
                                 func=mybir.ActivationFunctionType.Sqrt,
                                 bias=zero128)
            # q = dist * invpd  (invpd broadcast over t)
            q = acc.tile([128, NPAIR * T], FD)
            q3 = q[:, :].rearrange("p (i t) -> p i t", t=T)
            d3 = dist[:, :].rearrange("p (i t) -> p i t", t=T)
            nc.vector.tensor_tensor(
                out=q3, in0=d3,
                in1=bass.AP(tensor=invpd.tensor, offset=invpd.offset,
                            ap=[invpd.ap[0], [1, NPAIR], [0, T]]),
                op=mybir.AluOpType.mult)
            # pen_neg = min(q - 1, 0)
            nc.vector.tensor_scalar(out=q, in0=q, scalar1=1.0, scalar2=0.0,
                                    op0=mybir.AluOpType.subtract,
                                    op1=mybir.AluOpType.min)
            # weight (mask * ew / BNT) and reduce
            nc.vector.tensor_tensor(out=q, in0=q, in1=wtt,
                                    op=mybir.AluOpType.mult)
            part = acc.tile([128, 1], FD)
            nc.vector.tensor_reduce(out=part, in_=q,
                                    axis=mybir.AxisListType.X,
                                    op=mybir.AluOpType.add)
            nc.sync.dma_start(out=part_out, in_=part)

    nc.compile()
    return nc


def kernel(Y, length, width):
    Y = np.asarray(Y, np.float32)
    length = np.asarray(length, np.float32)
    width = np.asarray(width, np.float32)

    if "nc" not in _CACHE:
        _CACHE["nc"] = _build()
    nc = _CACHE["nc"]

    frac = (2.0 * np.arange(K, dtype=np.float32) / (K - 1) - 1.0).astype(np.float32)
    ew = DECAY_RATE ** np.arange(T, dtype=np.float32)
    ew = ew / ew.sum()

    in_maps = []
    for c in range(NCORES):
        b, h = divmod(c, 2)
        # rotate agents so i-agents are rows [0,32)
        perm = np.r_[h * 32:(h * 32 + N)] % N
        Yb = Y[b][perm].reshape(N, T * D)
        lb = length[b][perm].reshape(N, 1)
        wb = width[b][perm].reshape(N, 1)
        # weight tile: [128 = (parity, j), NPAIR*T]
        wt = np.zeros((128, NPAIR * T), np.float32)
        for p in range(128):
            ih, j = divmod(p, N)
            for ip in range(NPAIR):
                i_loc = 2 * ip + ih          # row in permuted agent axis
                if i_loc != j:               # mask diagonal
                    wt[p, ip * T:(ip + 1) * T] = ew / (B * N * T)
        in_maps.append({
            "y_in": Yb, "len_in": lb, "wid_in": wb,
            "frac_in": frac, "wt_in": wt,
        })

    global _LAST_INMAPS
    _LAST_INMAPS = in_maps
    res = bass_utils.run_bass_kernel_spmd(nc, in_maps,
                                          core_ids=list(range(NCORES)))
    total = 0.0
    for c in range(NCORES):
        total += float(res.results[c]["part_out"].astype(np.float64).sum())
    return np.float32(-total)



# revision 6
# speedup vs baseline: 1.5047x; 1.5047x over previous
"""AgentCollisionLoss Trainium2 kernel — PE quadratic-form formulation.

Sharding: 8 cores = B(4) x t-half(2). Core c: b = c//2, t in [40*(c%2), +40).

Math: d2[(j,l),(i,k),t] = sq_j(l) + sq_i(k) - 2(wx_j wx_i + wy_j wy_i)
computed as a contraction-20 fp32r matmul per (slab-pair, l):
  stationary (lhsT) [20, 128]: block-diag 2 slabs x 10 rows, cols (h,j)
  moving (rhs)      [20, 320]: 2 slabs x 10 rows, cols (i,k)
fp32r truncates inputs to ~12 mantissa bits but multiplies exactly, so
coords/sq are hi/lo split (hi = top-11-explicit-bit truncation via int
mask): sq_j*1 (2 rows) + 1*sq_i (2) + xh(-2xh), xl(-2xh), xh(-2xl) (3)
+ same for y (3) = 10 rows per slab. Dropped xl*xl terms ~3e-3 abs.

Drain: min over (l, k) via strided XY tensor_reduce; most pairs go
PSUM -ACT copy-> bf16 SBUF -> DVE 2x bf16 reduce; a few reduce direct
from PSUM on DVE to balance engines. Finish: sqrt (ACT), q=dist*invpd,
pen*w = relu(W - q*W), summed by ACT accum_out. Host sums 8x[128].
"""

import numpy as np

import concourse.bass as bass
import concourse.bacc as bacc
import concourse.tile as tile
import concourse.mybir as mybir
from concourse import bass_utils

B, N, T, D = 4, 64, 80, 6
K = 5
NCORES = 8
BUFFER_DIST = 0.2
DECAY_RATE = 0.9
TL = T // 2          # 40 slabs per core
NPAIR = TL // 2      # 20 slab-pairs
AK = N * K           # 320 (a,k) columns
FD = mybir.dt.float32
FR = mybir.dt.float32r
BF = mybir.dt.bfloat16
I32 = mybir.dt.int32
MASK = -4096         # 0xFFFFF000: truncate to 11 explicit mantissa bits
AF = mybir.ActivationFunctionType
AL = mybir.AluOpType

# slab-pairs drained directly from PSUM on DVE (rest: ACT bf16 + DVE)
DIRECT = (0, 10)

_CACHE = {}
_LAST_INMAPS = None


def _build():
    nc = bacc.Bacc("TRN2", target_bir_lowering=False, debug=False,
                   num_devices=NCORES)

    yt_in = nc.dram_tensor("yt_in", [TL, 3 * N], FD, kind="ExternalInput").ap()
    ck_in = nc.dram_tensor("ck_in", [AK], FD, kind="ExternalInput").ap()
    wm_in = nc.dram_tensor("wm_in", [128, NPAIR * N], FD,
                           kind="ExternalInput").ap()
    ip_in = nc.dram_tensor("ip_in", [128, N], FD, kind="ExternalInput").ap()
    z_in = nc.dram_tensor("z_in", [5 * NPAIR * 128], FR,
                          kind="ExternalInput").ap()
    part_out = nc.dram_tensor("part_out", [128, 1], FD,
                              kind="ExternalOutput").ap()

    SLW = 5 * NPAIR * 128    # SL free width (elements)

    with tile.TileContext(nc) as tc:
        with (
            tc.tile_pool(name="prep", bufs=1) as prep,
            tc.tile_pool(name="ops", bufs=1) as ops,
            tc.tile_pool(name="fin", bufs=1) as fin,
            tc.tile_pool(name="dtile", bufs=3) as dtile,
            tc.tile_pool(name="mtmp", bufs=3) as mtmp,
            tc.tile_pool(name="p3", bufs=2, space="PSUM") as p3pool,
            tc.tile_pool(name="p2", bufs=1, space="PSUM") as p2pool,
        ):
            # ---------- load ----------
            YT = prep.tile([TL, 3 * N], FD)
            nc.sync.dma_start(out=YT, in_=yt_in)
            CK = prep.tile([TL, AK], FD)
            nc.sync.dma_start(
                out=CK,
                in_=bass.AP(tensor=ck_in.tensor, offset=0,
                            ap=[[0, TL], [1, AK]]))
            WM = fin.tile([128, NPAIR * N], FD)
            nc.sync.dma_start(out=WM, in_=wm_in)
            IP = fin.tile([128, N], FD)
            nc.sync.dma_start(out=IP, in_=ip_in)

            def colblk(t, c0, n):
                return bass.AP(tensor=t.tensor, offset=t.offset + c0,
                               ap=[t.ap[0], [1, n]])

            def bcast_ak(t, c0):     # [TL, 64] col-block -> (a,k) bcast view
                return bass.AP(tensor=t.tensor, offset=t.offset + c0,
                               ap=[t.ap[0], [1, N], [0, K]])

            pi2 = prep.tile([TL, 1], FD)
            nc.vector.memset(pi2, float(np.pi / 2))
            zb = prep.tile([TL, 1], FD)
            nc.vector.memset(zb, 0.0)

            cosT = prep.tile([TL, N], FD)
            sinT = prep.tile([TL, N], FD)
            yaw_ap = colblk(YT, 2 * N, N)
            nc.scalar.activation(out=cosT, in_=yaw_ap, func=AF.Sin, bias=pi2)
            nc.scalar.activation(out=sinT, in_=yaw_ap, func=AF.Sin, bias=zb)

            # ---------- world disk coords [TL, (a,k)] ----------
            wx = prep.tile([TL, AK], FD)
            wy = prep.tile([TL, AK], FD)
            tmp = prep.tile([TL, AK], FD)
            tmq = prep.tile([TL, AK], FD)
            nc.vector.tensor_tensor(out=tmp, in0=CK, in1=bcast_ak(cosT, 0),
                                    op=AL.mult)
            nc.vector.tensor_tensor(out=wx, in0=tmp, in1=bcast_ak(YT, 0),
                                    op=AL.add)
            nc.vector.tensor_tensor(out=tmq, in0=CK, in1=bcast_ak(sinT, 0),
                                    op=AL.mult)
            nc.vector.tensor_tensor(out=wy, in0=bcast_ak(YT, N), in1=tmq,
                                    op=AL.subtract)

            s1 = prep.tile([TL, AK], FD)
            s2 = prep.tile([TL, AK], FD)
            nc.scalar.activation(out=s1, in_=wx, func=AF.Square, bias=zb)
            nc.scalar.activation(out=s2, in_=wy, func=AF.Square, bias=zb)
            sq = prep.tile([TL, AK], FD)
            nc.gpsimd.tensor_tensor(out=sq, in0=s1, in1=s2, op=AL.add)

            # ---------- hi/lo splits ----------
            def hi_split(src, name):
                h = prep.tile([TL, AK], FD, tag=name + "h")
                nc.vector.tensor_scalar(
                    out=h.bitcast(I32), in0=src.bitcast(I32),
                    scalar1=MASK, scalar2=None, op0=AL.bitwise_and)
                lo = prep.tile([TL, AK], FD, tag=name + "l")
                nc.gpsimd.tensor_tensor(out=lo, in0=src, in1=h,
                                        op=AL.subtract)
                return h, lo

            xh, xl = hi_split(wx, "x")
            yh, yl = hi_split(wy, "y")
            sqh, sqb = hi_split(sq, "s")

            def scale_m2(src, name):
                d = prep.tile([TL, AK], FD, tag=name)
                nc.vector.tensor_scalar(out=d, in0=src, scalar1=-2.0,
                                        scalar2=0.0, op0=AL.mult, op1=AL.add)
                return d

            m2xh = scale_m2(xh, "m2xh")
            m2xl = scale_m2(xl, "m2xl")
            m2yh = scale_m2(yh, "m2yh")
            m2yl = scale_m2(yl, "m2yl")

            onesA = prep.tile([TL, AK], FD)
            nc.vector.memset(onesA, 1.0)

            # ---------- (l,a) reorder for stationary side ----------
            def lreorder(src, name):
                d = prep.tile([TL, AK], FD, tag=name)
                nc.vector.tensor_copy(
                    bass.AP(tensor=d.tensor, offset=d.offset,
                            ap=[d.ap[0], [N, K], [1, N]]),
                    bass.AP(tensor=src.tensor, offset=src.offset,
                            ap=[src.ap[0], [1, K], [K, N]]))
                return d

            sqhL = lreorder(sqh, "sqhL")
            sqbL = lreorder(sqb, "sqbL")
            xhL = lreorder(xh, "xhL")
            xlL = lreorder(xl, "xlL")
            yhL = lreorder(yh, "yhL")
            ylL = lreorder(yl, "ylL")

            # ---------- operand assembly (via DRAM images) ----------
            # SL [20, 5*20*128] fp32r: col = l*2560 + p*128 + h*64 + j
            sl_img = nc.dram_tensor("sl_img", [20 * SLW], FR,
                                    kind="Internal").ap()
            v_img = nc.dram_tensor("v_img", [20 * NPAIR * AK], FR,
                                   kind="Internal").ap()

            # zero-fill the stationary image (block-diag zeros)
            nc.sync.dma_start(
                out=bass.AP(tensor=sl_img.tensor, offset=0,
                            ap=[[SLW, 20], [1, SLW]]),
                in_=bass.AP(tensor=z_in.tensor, offset=0,
                            ap=[[0, 20], [1, SLW]]))

            S_ROWS = [sqhL, sqbL, onesA, onesA, xhL, xlL, xhL, yhL, ylL, yhL]
            M_ROWS = [onesA, onesA, sqh, sqb, m2xh, m2xh, m2xl,
                      m2yh, m2yh, m2yl]

            for h in range(2):
                for r in range(10):
                    srcS = S_ROWS[r][h * NPAIR:(h + 1) * NPAIR, :]
                    nc.sync.dma_start(
                        out=bass.AP(tensor=sl_img.tensor,
                                    offset=(h * 10 + r) * SLW + h * N,
                                    ap=[[128, NPAIR], [2560, 5], [1, N]]),
                        in_=bass.AP(tensor=srcS.tensor, offset=srcS.offset,
                                    ap=[srcS.ap[0], [N, 5],
                                        [1, N]]).bitcast(FR))
                    srcM = M_ROWS[r][h * NPAIR:(h + 1) * NPAIR, :]
                    nc.sync.dma_start(
                        out=bass.AP(tensor=v_img.tensor,
                                    offset=(h * 10 + r) * NPAIR * AK,
                                    ap=[[AK, NPAIR], [1, AK]]),
                        in_=bass.AP(tensor=srcM.tensor, offset=srcM.offset,
                                    ap=[srcM.ap[0], [1, AK]]).bitcast(FR))

            SL = ops.tile([20, SLW], FR)
            nc.sync.dma_start(
                out=SL,
                in_=bass.AP(tensor=sl_img.tensor, offset=0,
                            ap=[[SLW, 20], [1, SLW]]))
            V = ops.tile([20, NPAIR * AK], FR)
            nc.sync.dma_start(
                out=V,
                in_=bass.AP(tensor=v_img.tensor, offset=0,
                            ap=[[NPAIR * AK, 20], [1, NPAIR * AK]]))

            # ---------- main loop ----------
            dmin2 = fin.tile([128, NPAIR * N], BF)

            for p in range(NPAIR):
                P3 = p3pool.tile([128, 3 * 512], FD, tag="P3")
                P2 = p2pool.tile([128, 2 * 512], FD, tag="P2")
                for l in range(5):
                    dst = P3 if l < 3 else P2
                    c0 = 512 * l if l < 3 else 512 * (l - 3)
                    nc.tensor.matmul(
                        out=dst[0:128, c0:c0 + AK],
                        lhsT=SL[0:20, 2560 * l + 128 * p:
                                2560 * l + 128 * p + 128],
                        rhs=V[0:20, AK * p:AK * (p + 1)],
                        tile_position=(0, 0))

                dslice = bass.AP(tensor=dmin2.tensor,
                                 offset=dmin2.offset + p * N,
                                 ap=[dmin2.ap[0], [1, N]])
                if p in DIRECT:
                    m3 = mtmp.tile([128, N], FD, tag="m3")
                    m2t = mtmp.tile([128, N], FD, tag="m2t")
                    nc.vector.tensor_reduce(
                        out=m3,
                        in_=bass.AP(tensor=P3.tensor, offset=P3.offset,
                                    ap=[P3.ap[0], [K, N], [512, 3], [1, K]]),
                        axis=mybir.AxisListType.XY, op=AL.min)
                    nc.vector.tensor_reduce(
                        out=m2t,
                        in_=bass.AP(tensor=P2.tensor, offset=P2.offset,
                                    ap=[P2.ap[0], [K, N], [512, 2], [1, K]]),
                        axis=mybir.AxisListType.XY, op=AL.min)
                    nc.vector.tensor_tensor(out=dslice, in0=m3, in1=m2t,
                                            op=AL.min)
                else:
                    Dt = dtile.tile([128, 5 * AK], BF, tag="D")
                    nc.scalar.activation(
                        out=bass.AP(tensor=Dt.tensor, offset=Dt.offset,
                                    ap=[Dt.ap[0], [AK, 3], [1, AK]]),
                        in_=bass.AP(tensor=P3.tensor, offset=P3.offset,
                                    ap=[P3.ap[0], [512, 3], [1, AK]]),
                        func=AF.Copy)
                    nc.scalar.activation(
                        out=bass.AP(tensor=Dt.tensor,
                                    offset=Dt.offset + 3 * AK,
                                    ap=[Dt.ap[0], [AK, 2], [1, AK]]),
                        in_=bass.AP(tensor=P2.tensor, offset=P2.offset,
                                    ap=[P2.ap[0], [512, 2], [1, AK]]),
                        func=AF.Copy)
                    nc.vector.tensor_reduce(
                        out=dslice,
                        in_=bass.AP(tensor=Dt.tensor, offset=Dt.offset,
                                    ap=[Dt.ap[0], [K, N], [AK, 5], [1, K]]),
                        axis=mybir.AxisListType.XY, op=AL.min)

            # ---------- finish ----------
            nc.vector.tensor_scalar(out=dmin2, in0=dmin2, scalar1=0.0,
                                    scalar2=None, op0=AL.max)
            zb128 = fin.tile([128, 1], FD)
            nc.vector.memset(zb128, 0.0)
            dist = fin.tile([128, NPAIR * N], FD)
            nc.scalar.activation(out=dist, in_=dmin2, func=AF.Sqrt,
                                 bias=zb128)
            q = fin.tile([128, NPAIR * N], FD)
            nc.vector.tensor_tensor(
                out=bass.AP(tensor=q.tensor, offset=q.offset,
                            ap=[q.ap[0], [N, NPAIR], [1, N]]),
                in0=bass.AP(tensor=dist.tensor, offset=dist.offset,
                            ap=[dist.ap[0], [N, NPAIR], [1, N]]),
                in1=bass.AP(tensor=IP.tensor, offset=IP.offset,
                            ap=[IP.ap[0], [0, NPAIR], [1, N]]),
                op=AL.mult)
            t2 = fin.tile([128, NPAIR * N], FD)
            nc.vector.tensor_tensor(out=t2, in0=q, in1=WM, op=AL.mult)
            u = fin.tile([128, NPAIR * N], FD)
            nc.vector.tensor_tensor(out=u, in0=WM, in1=t2, op=AL.subtract)
            part = fin.tile([128, 1], FD)
            nc.vector.memset(part, 0.0)
            nc.scalar.activation(out=u, in_=u, func=AF.Relu, bias=zb128,
                                 accum_out=part)
            nc.sync.dma_start(out=part_out, in_=part)

    nc.compile()
    return nc


def kernel(Y, length, width):
    Y = np.asarray(Y, np.float32)
    length = np.asarray(length, np.float32)
    width = np.asarray(width, np.float32)

    if "nc" not in _CACHE:
        _CACHE["nc"] = _build()
    nc = _CACHE["nc"]

    f2 = (2.0 * np.arange(K, dtype=np.float32) / (K - 1) - 1.0)
    ew = DECAY_RATE ** np.arange(T, dtype=np.float32)
    ew = (ew / ew.sum()).astype(np.float64)

    # prep-row rr = h*20 + p  <->  local slab t_local = 2p + h
    rr = np.arange(TL)
    tl_of_rr = 2 * (rr % NPAIR) + rr // NPAIR

    in_maps = []
    for c in range(NCORES):
        b, th = divmod(c, 2)
        t0 = th * TL
        tglob = t0 + tl_of_rr                       # [TL] global t per row

        yt = np.empty((TL, 3 * N), np.float32)
        yt[:, 0:N] = Y[b, :, tglob, 0]              # x[t, a]
        yt[:, N:2 * N] = Y[b, :, tglob, 1]          # y
        yt[:, 2 * N:3 * N] = Y[b, :, tglob, 4]      # yaw

        rad = width[b] / 2.0
        cmax = length[b] / 2.0 - rad                # [N]
        ck = (cmax[:, None] * f2[None, :]).reshape(AK).astype(np.float32)

        pd = rad[:, None] + rad[None, :] + BUFFER_DIST   # [j, i]
        ip = np.empty((128, N), np.float32)
        ip[0:N] = 1.0 / pd
        ip[N:128] = 1.0 / pd

        wm = np.zeros((128, NPAIR * N), np.float64)
        mask = (~np.eye(N, dtype=bool)).astype(np.float64)   # [j, i]
        for p in range(NPAIR):
            for h in range(2):
                t = t0 + 2 * p + h
                wm[h * N:(h + 1) * N, p * N:(p + 1) * N] = \
                    mask * (ew[t] / (B * N * T))
        wm = wm.astype(np.float32)

        in_maps.append({
            "yt_in": yt, "ck_in": ck, "wm_in": wm, "ip_in": ip,
            "z_in": np.zeros(5 * NPAIR * 128, np.float32),
        })

    global _LAST_INMAPS
    _LAST_INMAPS = in_maps
    res = bass_utils.run_bass_kernel_spmd(nc, in_maps,
                                          core_ids=list(range(NCORES)))
    total = 0.0
    for c in range(NCORES):
        total += float(res.results[c]["part_out"].astype(np.float64).sum())
    return np.float32(total)


# revision 7
# speedup vs baseline: 1.9557x; 1.2997x over previous
"""AgentCollisionLoss Trainium2 kernel — PE quadratic-form formulation.

Sharding: 8 cores = B(4) x t-half(2). Core c: b = c//2, t in [40*(c%2), +40).

Math: d2[(j),(i,k),t,l] = sq_j(l) + sq_i(k) - 2(wx_j wx_i + wy_j wy_i),
one bf16 matmul per (slab-pair, l):
  stationary (lhsT) [28, 128]: block-diag 2 slabs x 14 rows, cols (h,j)
  moving (rhs)      [28, 320]: 2 slabs x 14 rows, cols (i,k)

Precision: coords are represented 2-way in bf16 (xa = bf16(wx),
xb = bf16(wx - xa); residual ~3e-4 enters d2 via 2*dx*eps — negligible
near collisions), sq is computed FROM the truncated coords (so the
quadratic form is exactly (x~_i - x~_j)^2 + ...) and split 3-way in
bf16 (residual 2e-4). All bf16 products are exact in the fp32 PSUM
accumulator. 14 rows/slab: 3 sq_j + 3 sq_i + 4 x-cross + 4 y-cross.

Drain: ACT copies PSUM->bf16 SBUF in (i, [l,k] padded to 26) layout,
DVE 2x bf16 X-reduce min over 26; a few pairs reduce directly from
PSUM on DVE to balance engines. Finish: sqrt (ACT), q = dist*invpd,
pen*w = relu(W - q*W) summed by ACT accum_out. Host sums 8x[128].
"""

import numpy as np
import ml_dtypes

import concourse.bass as bass
import concourse.bacc as bacc
import concourse.tile as tile
import concourse.mybir as mybir
from concourse import bass_utils

B, N, T, D = 4, 64, 80, 6
K = 5
NCORES = 8
BUFFER_DIST = 0.2
DECAY_RATE = 0.9
TL = T // 2          # 40 slabs per core
NPAIR = TL // 2      # 20 slab-pairs
AK = N * K           # 320 (a,k) columns
KL = 26              # padded (l,k) group for even-dim 2x reduce
FD = mybir.dt.float32
BF = mybir.dt.bfloat16
AF = mybir.ActivationFunctionType
AL = mybir.AluOpType
NR = 14              # contraction rows per slab

# slab-pairs drained directly from PSUM on DVE (rest: ACT bf16 + DVE)
DIRECT = (0, 10)

_CACHE = {}
_LAST_INMAPS = None


def _build():
    nc = bacc.Bacc("TRN2", target_bir_lowering=False, debug=False,
                   num_devices=NCORES)

    yt_in = nc.dram_tensor("yt_in", [TL, 3 * N], FD, kind="ExternalInput").ap()
    ck_in = nc.dram_tensor("ck_in", [AK], FD, kind="ExternalInput").ap()
    wm_in = nc.dram_tensor("wm_in", [128, NPAIR * N], FD,
                           kind="ExternalInput").ap()
    ip_in = nc.dram_tensor("ip_in", [128, N], FD, kind="ExternalInput").ap()
    SLW = 5 * NPAIR * 128    # SL free width (elements)
    z_in = nc.dram_tensor("z_in", [SLW], BF, kind="ExternalInput").ap()
    part_out = nc.dram_tensor("part_out", [128, 1], FD,
                              kind="ExternalOutput").ap()

    with tile.TileContext(nc) as tc:
        with (
            tc.tile_pool(name="prep", bufs=1) as prep,
            tc.tile_pool(name="ops", bufs=1) as ops,
            tc.tile_pool(name="fin", bufs=1) as fin,
            tc.tile_pool(name="dtile", bufs=3) as dtile,
            tc.tile_pool(name="mtmp", bufs=3) as mtmp,
            tc.tile_pool(name="p3", bufs=2, space="PSUM") as p3pool,
            tc.tile_pool(name="p2", bufs=1, space="PSUM") as p2pool,
        ):
            # ---------- load ----------
            YT = prep.tile([TL, 3 * N], FD)
            nc.sync.dma_start(out=YT, in_=yt_in)
            CK = prep.tile([TL, AK], FD)
            nc.sync.dma_start(
                out=CK,
                in_=bass.AP(tensor=ck_in.tensor, offset=0,
                            ap=[[0, TL], [1, AK]]))
            WM = fin.tile([128, NPAIR * N], FD)
            nc.sync.dma_start(out=WM, in_=wm_in)
            IP = fin.tile([128, N], FD)
            nc.sync.dma_start(out=IP, in_=ip_in)

            def colblk(t, c0, n):
                return bass.AP(tensor=t.tensor, offset=t.offset + c0,
                               ap=[t.ap[0], [1, n]])

            def bcast_ak(t, c0):     # [TL, 64] col-block -> (a,k) bcast view
                return bass.AP(tensor=t.tensor, offset=t.offset + c0,
                               ap=[t.ap[0], [1, N], [0, K]])

            pi2 = prep.tile([TL, 1], FD)
            nc.vector.memset(pi2, float(np.pi / 2))
            zb = prep.tile([TL, 1], FD)
            nc.vector.memset(zb, 0.0)

            cosT = prep.tile([TL, N], FD)
            sinT = prep.tile([TL, N], FD)
            yaw_ap = colblk(YT, 2 * N, N)
            nc.scalar.activation(out=cosT, in_=yaw_ap, func=AF.Sin, bias=pi2)
            nc.scalar.activation(out=sinT, in_=yaw_ap, func=AF.Sin, bias=zb)

            # ---------- world disk coords [TL, (a,k)] ----------
            wx = prep.tile([TL, AK], FD)
            wy = prep.tile([TL, AK], FD)
            tmp = prep.tile([TL, AK], FD)
            tmq = prep.tile([TL, AK], FD)
            nc.vector.tensor_tensor(out=tmp, in0=CK, in1=bcast_ak(cosT, 0),
                                    op=AL.mult)
            nc.vector.tensor_tensor(out=wx, in0=tmp, in1=bcast_ak(YT, 0),
                                    op=AL.add)
            nc.vector.tensor_tensor(out=tmq, in0=CK, in1=bcast_ak(sinT, 0),
                                    op=AL.mult)
            nc.vector.tensor_tensor(out=wy, in0=bcast_ak(YT, N), in1=tmq,
                                    op=AL.subtract)

            # ---------- bf16 2-way coord split ----------
            def split2(src, name):
                a = prep.tile([TL, AK], BF, tag=name + "a")
                nc.vector.tensor_copy(a, src)
                r = prep.tile([TL, AK], FD, tag=name + "r")
                nc.gpsimd.tensor_tensor(out=r, in0=src, in1=a,
                                        op=AL.subtract)
                b = prep.tile([TL, AK], BF, tag=name + "b")
                nc.vector.tensor_copy(b, r)
                # xt = a + b (truncated coord, fp32)
                xt = prep.tile([TL, AK], FD, tag=name + "t")
                nc.gpsimd.tensor_tensor(out=xt, in0=src, in1=r, op=AL.subtract)
                nc.vector.tensor_tensor(out=xt, in0=xt, in1=b, op=AL.add)
                return a, b, xt

            xa, xb, xt = split2(wx, "x")
            ya, yb, yt = split2(wy, "y")

            # sq from truncated coords, 3-way bf16 split
            s1 = prep.tile([TL, AK], FD)
            s2 = prep.tile([TL, AK], FD)
            nc.scalar.activation(out=s1, in_=xt, func=AF.Square, bias=zb)
            nc.scalar.activation(out=s2, in_=yt, func=AF.Square, bias=zb)
            sq = prep.tile([TL, AK], FD)
            nc.gpsimd.tensor_tensor(out=sq, in0=s1, in1=s2, op=AL.add)

            sa = prep.tile([TL, AK], BF)
            nc.vector.tensor_copy(sa, sq)
            r2 = prep.tile([TL, AK], FD)
            nc.gpsimd.tensor_tensor(out=r2, in0=sq, in1=sa, op=AL.subtract)
            sb = prep.tile([TL, AK], BF)
            nc.vector.tensor_copy(sb, r2)
            r3 = prep.tile([TL, AK], FD)
            nc.gpsimd.tensor_tensor(out=r3, in0=r2, in1=sb, op=AL.subtract)
            sc = prep.tile([TL, AK], BF)
            nc.vector.tensor_copy(sc, r3)

            def scale_m2(src, name):
                d = prep.tile([TL, AK], BF, tag=name)
                nc.vector.tensor_scalar(out=d, in0=src, scalar1=-2.0,
                                        scalar2=0.0, op0=AL.mult, op1=AL.add)
                return d

            m2xa = scale_m2(xa, "m2xa")
            m2xb = scale_m2(xb, "m2xb")
            m2ya = scale_m2(ya, "m2ya")
            m2yb = scale_m2(yb, "m2yb")

            onesA = prep.tile([TL, AK], BF)
            nc.vector.memset(onesA, 1.0)

            # ---------- (l,a) reorder for stationary side ----------
            def lreorder(src, name):
                d = prep.tile([TL, AK], BF, tag=name)
                nc.vector.tensor_copy(
                    bass.AP(tensor=d.tensor, offset=d.offset,
                            ap=[d.ap[0], [N, K], [1, N]]),
                    bass.AP(tensor=src.tensor, offset=src.offset,
                            ap=[src.ap[0], [1, K], [K, N]]))
                return d

            saL = lreorder(sa, "saL")
            sbL = lreorder(sb, "sbL")
            scL = lreorder(sc, "scL")
            xaL = lreorder(xa, "xaL")
            xbL = lreorder(xb, "xbL")
            yaL = lreorder(ya, "yaL")
            ybL = lreorder(yb, "ybL")

            # ---------- operand assembly (via DRAM images) ----------
            # SL [28, 5*20*128] bf16: col = l*2560 + p*128 + h*64 + j
            sl_img = nc.dram_tensor("sl_img", [2 * NR * SLW], BF,
                                    kind="Internal").ap()
            v_img = nc.dram_tensor("v_img", [2 * NR * NPAIR * AK], BF,
                                   kind="Internal").ap()

            nc.scalar.dma_start(
                out=bass.AP(tensor=sl_img.tensor, offset=0,
                            ap=[[SLW, 2 * NR], [1, SLW]]),
                in_=bass.AP(tensor=z_in.tensor, offset=0,
                            ap=[[0, 2 * NR], [1, SLW]]))

            S_ROWS = [saL, sbL, scL, onesA, onesA, onesA,
                      xaL, xaL, xbL, xbL, yaL, yaL, ybL, ybL]
            M_ROWS = [onesA, onesA, onesA, sa, sb, sc,
                      m2xa, m2xb, m2xa, m2xb, m2ya, m2yb, m2ya, m2yb]

            for h in range(2):
                for r in range(NR):
                    srcS = S_ROWS[r][h * NPAIR:(h + 1) * NPAIR, :]
                    nc.scalar.dma_start(
                        out=bass.AP(tensor=sl_img.tensor,
                                    offset=(h * NR + r) * SLW + h * N,
                                    ap=[[128, NPAIR], [2560, 5], [1, N]]),
                        in_=bass.AP(tensor=srcS.tensor, offset=srcS.offset,
                                    ap=[srcS.ap[0], [N, 5], [1, N]]))
                    srcM = M_ROWS[r][h * NPAIR:(h + 1) * NPAIR, :]
                    nc.sync.dma_start(
                        out=bass.AP(tensor=v_img.tensor,
                                    offset=(h * NR + r) * NPAIR * AK,
                                    ap=[[AK, NPAIR], [1, AK]]),
                        in_=bass.AP(tensor=srcM.tensor, offset=srcM.offset,
                                    ap=[srcM.ap[0], [1, AK]]))

            SL = ops.tile([2 * NR, SLW], BF)
            nc.scalar.dma_start(
                out=SL,
                in_=bass.AP(tensor=sl_img.tensor, offset=0,
                            ap=[[SLW, 2 * NR], [1, SLW]]))
            V = ops.tile([2 * NR, NPAIR * AK], BF)
            nc.sync.dma_start(
                out=V,
                in_=bass.AP(tensor=v_img.tensor, offset=0,
                            ap=[[NPAIR * AK, 2 * NR], [1, NPAIR * AK]]))

            # ---------- main loop ----------
            dmin2 = fin.tile([128, NPAIR * N], BF)

            for p in range(NPAIR):
                P3 = p3pool.tile([128, 3 * 512], FD, tag="P3")
                P2 = p2pool.tile([128, 2 * 512], FD, tag="P2")
                for l in range(5):
                    dst = P3 if l < 3 else P2
                    c0 = 512 * l if l < 3 else 512 * (l - 3)
                    nc.tensor.matmul(
                        out=dst[0:128, c0:c0 + AK],
                        lhsT=SL[0:2 * NR, 2560 * l + 128 * p:
                                2560 * l + 128 * p + 128],
                        rhs=V[0:2 * NR, AK * p:AK * (p + 1)],
                        tile_position=(0, 0))

                dslice = bass.AP(tensor=dmin2.tensor,
                                 offset=dmin2.offset + p * N,
                                 ap=[dmin2.ap[0], [1, N]])
                if p in DIRECT:
                    m3 = mtmp.tile([128, N], FD, tag="m3")
                    m2t = mtmp.tile([128, N], FD, tag="m2t")
                    nc.vector.tensor_reduce(
                        out=m3,
                        in_=bass.AP(tensor=P3.tensor, offset=P3.offset,
                                    ap=[P3.ap[0], [K, N], [512, 3], [1, K]]),
                        axis=mybir.AxisListType.XY, op=AL.min)
                    nc.vector.tensor_reduce(
                        out=m2t,
                        in_=bass.AP(tensor=P2.tensor, offset=P2.offset,
                                    ap=[P2.ap[0], [K, N], [512, 2], [1, K]]),
                        axis=mybir.AxisListType.XY, op=AL.min)
                    nc.vector.tensor_tensor(out=dslice, in0=m3, in1=m2t,
                                            op=AL.min)
                else:
                    # D layout: col = i*26 + l*5 + k, col 25 = +big pad
                    Dt = dtile.tile([128, N * KL], BF, tag="D")
                    nc.vector.memset(
                        bass.AP(tensor=Dt.tensor, offset=Dt.offset + KL - 1,
                                ap=[Dt.ap[0], [KL, N]]), 1e30)
                    nc.scalar.activation(
                        out=bass.AP(tensor=Dt.tensor, offset=Dt.offset,
                                    ap=[Dt.ap[0], [K, 3], [KL, N], [1, K]]),
                        in_=bass.AP(tensor=P3.tensor, offset=P3.offset,
                                    ap=[P3.ap[0], [512, 3], [K, N], [1, K]]),
                        func=AF.Copy)
                    nc.scalar.activation(
                        out=bass.AP(tensor=Dt.tensor,
                                    offset=Dt.offset + 3 * K,
                                    ap=[Dt.ap[0], [K, 2], [KL, N], [1, K]]),
                        in_=bass.AP(tensor=P2.tensor, offset=P2.offset,
                                    ap=[P2.ap[0], [512, 2], [K, N], [1, K]]),
                        func=AF.Copy)
                    nc.vector.tensor_reduce(
                        out=dslice,
                        in_=bass.AP(tensor=Dt.tensor, offset=Dt.offset,
                                    ap=[Dt.ap[0], [KL, N], [1, KL]]),
                        axis=mybir.AxisListType.X, op=AL.min)

            # ---------- finish ----------
            nc.vector.tensor_scalar(out=dmin2, in0=dmin2, scalar1=0.0,
                                    scalar2=None, op0=AL.max)
            zb128 = fin.tile([128, 1], FD)
            nc.vector.memset(zb128, 0.0)
            dist = fin.tile([128, NPAIR * N], FD)
            nc.scalar.activation(out=dist, in_=dmin2, func=AF.Sqrt,
                                 bias=zb128)
            q = fin.tile([128, NPAIR * N], FD)
            nc.vector.tensor_tensor(
                out=bass.AP(tensor=q.tensor, offset=q.offset,
                            ap=[q.ap[0], [N, NPAIR], [1, N]]),
                in0=bass.AP(tensor=dist.tensor, offset=dist.offset,
                            ap=[dist.ap[0], [N, NPAIR], [1, N]]),
                in1=bass.AP(tensor=IP.tensor, offset=IP.offset,
                            ap=[IP.ap[0], [0, NPAIR], [1, N]]),
                op=AL.mult)
            t2 = fin.tile([128, NPAIR * N], FD)
            nc.vector.tensor_tensor(out=t2, in0=q, in1=WM, op=AL.mult)
            u = fin.tile([128, NPAIR * N], FD)
            nc.vector.tensor_tensor(out=u, in0=WM, in1=t2, op=AL.subtract)
            part = fin.tile([128, 1], FD)
            nc.vector.memset(part, 0.0)
            nc.scalar.activation(out=u, in_=u, func=AF.Relu, bias=zb128,
                                 accum_out=part)
            nc.sync.dma_start(out=part_out, in_=part)

    nc.compile()
    return nc


def kernel(Y, length, width):
    Y = np.asarray(Y, np.float32)
    length = np.asarray(length, np.float32)
    width = np.asarray(width, np.float32)

    if "nc" not in _CACHE:
        _CACHE["nc"] = _build()
    nc = _CACHE["nc"]

    f2 = (2.0 * np.arange(K, dtype=np.float32) / (K - 1) - 1.0)
    ew = DECAY_RATE ** np.arange(T, dtype=np.float32)
    ew = (ew / ew.sum()).astype(np.float64)

    # prep-row rr = h*20 + p  <->  local slab t_local = 2p + h
    rr = np.arange(TL)
    tl_of_rr = 2 * (rr % NPAIR) + rr // NPAIR

    in_maps = []
    for c in range(NCORES):
        b, th = divmod(c, 2)
        t0 = th * TL
        tglob = t0 + tl_of_rr                       # [TL] global t per row

        yt = np.empty((TL, 3 * N), np.float32)
        yt[:, 0:N] = Y[b, :, tglob, 0]              # x[t, a]
        yt[:, N:2 * N] = Y[b, :, tglob, 1]          # y
        yt[:, 2 * N:3 * N] = Y[b, :, tglob, 4]      # yaw

        rad = width[b] / 2.0
        cmax = length[b] / 2.0 - rad                # [N]
        ck = (cmax[:, None] * f2[None, :]).reshape(AK).astype(np.float32)

        pd = rad[:, None] + rad[None, :] + BUFFER_DIST   # [j, i]
        ip = np.empty((128, N), np.float32)
        ip[0:N] = 1.0 / pd
        ip[N:128] = 1.0 / pd

        wm = np.zeros((128, NPAIR * N), np.float64)
        mask = (~np.eye(N, dtype=bool)).astype(np.float64)   # [j, i]
        for p in range(NPAIR):
            for h in range(2):
                t = t0 + 2 * p + h
                wm[h * N:(h + 1) * N, p * N:(p + 1) * N] = \
                    mask * (ew[t] / (B * N * T))
        wm = wm.astype(np.float32)

        in_maps.append({
            "yt_in": yt, "ck_in": ck, "wm_in": wm, "ip_in": ip,
            "z_in": np.zeros(5 * NPAIR * 128, ml_dtypes.bfloat16),
        })

    global _LAST_INMAPS
    _LAST_INMAPS = in_maps
    res = bass_utils.run_bass_kernel_spmd(nc, in_maps,
                                          core_ids=list(range(NCORES)))
    total = 0.0
    for c in range(NCORES):
        total += float(res.results[c]["part_out"].astype(np.float64).sum())
    return np.float32(total)


# revision 9
# speedup vs baseline: 1.9668x; 1.0056x over previous
"""AgentCollisionLoss Trainium2 kernel — PE quadratic-form formulation.

Sharding: 8 cores = B(4) x t-half(2). Core c: b = c//2, t in [40*(c%2), +40).

d2[(j),(i,k),t,l] = sq_j(l) + sq_i(k) - 2(wx_j wx_i + wy_j wy_i), one bf16
matmul per (slab-pair, l): stationary [28, 128] block-diag 2 slabs x 14
rows, cols (h,j); moving [28, 320] cols (i,k). Coords 2-way bf16 split
(residual enters d2 as 2*dx*eps), sq computed FROM truncated coords and
split 3-way; all bf16 products exact in fp32 PSUM.

Drain: ACT copies PSUM->flat bf16 D [128,(l,i,k)]; min over (l,k) by
DVE XY-reduce (most pairs), or GPSIMD TT-min tree + DVE k-reduce, or
direct DVE PSUM reduce — mix balances engines. Finish interleaved in
2 chunks: clamp, sqrt (ACT), t2 = dist*WI (WI = W*invpd host-folded),
u = W - t2, relu+accum_out. Host sums 8 x [128, 2].
"""

import numpy as np
import ml_dtypes

import concourse.bass as bass
import concourse.bacc as bacc
import concourse.tile as tile
import concourse.mybir as mybir
from concourse import bass_utils

B, N, T, D = 4, 64, 80, 6
K = 5
NCORES = 8
BUFFER_DIST = 0.2
DECAY_RATE = 0.9
TL = T // 2          # 40 slabs per core
NPAIR = TL // 2      # 20 slab-pairs
AK = N * K           # 320 (a,k) columns
FD = mybir.dt.float32
BF = mybir.dt.bfloat16
AF = mybir.ActivationFunctionType
AL = mybir.AluOpType
NR = 14              # contraction rows per slab
SLW = NPAIR * 640    # SL cols: p*640 + l*128 + h*64 + j

DIRECT = (0, 10)             # drain directly from PSUM on DVE
GPSD = ()                    # GPSIMD TT-min drain (bf16 TT rejected on Pool)

_CACHE = {}
_LAST_INMAPS = None


def _build():
    nc = bacc.Bacc("TRN2", target_bir_lowering=False, debug=False,
                   num_devices=NCORES)

    yt_in = nc.dram_tensor("yt_in", [TL, 3 * N], FD, kind="ExternalInput").ap()
    ck_in = nc.dram_tensor("ck_in", [AK], FD, kind="ExternalInput").ap()
    wm_in = nc.dram_tensor("wm_in", [128, NPAIR * N], FD,
                           kind="ExternalInput").ap()
    wi_in = nc.dram_tensor("wi_in", [128, NPAIR * N], FD,
                           kind="ExternalInput").ap()
    z_in = nc.dram_tensor("z_in", [SLW], BF, kind="ExternalInput").ap()
    part_out = nc.dram_tensor("part_out", [128, 2], FD,
                              kind="ExternalOutput").ap()

    with tile.TileContext(nc) as tc:
        with (
            tc.tile_pool(name="prep", bufs=1) as prep,
            tc.tile_pool(name="ops", bufs=1) as ops,
            tc.tile_pool(name="fin", bufs=1) as fin,
            tc.tile_pool(name="dtile", bufs=3) as dtile,
            tc.tile_pool(name="mtmp", bufs=3) as mtmp,
            tc.tile_pool(name="gt", bufs=2) as gtp,
            tc.tile_pool(name="p3", bufs=2, space="PSUM") as p3pool,
            tc.tile_pool(name="p2", bufs=1, space="PSUM") as p2pool,
        ):
            # SL zero-fill first — depends on nothing, runs during prep
            SL = ops.tile([2 * NR, SLW], BF)
            nc.scalar.dma_start(
                out=bass.AP(tensor=SL.tensor, offset=SL.offset,
                            ap=[SL.ap[0], [1, SLW]]),
                in_=bass.AP(tensor=z_in.tensor, offset=0,
                            ap=[[0, 2 * NR], [1, SLW]]))
            V = ops.tile([2 * NR, NPAIR * AK], BF)

            # ---------- load ----------
            YT = prep.tile([TL, 3 * N], FD)
            nc.sync.dma_start(out=YT, in_=yt_in)
            CK = prep.tile([TL, AK], FD)
            nc.sync.dma_start(
                out=CK,
                in_=bass.AP(tensor=ck_in.tensor, offset=0,
                            ap=[[0, TL], [1, AK]]))
            WM = fin.tile([128, NPAIR * N], FD)
            nc.sync.dma_start(out=WM, in_=wm_in)
            WI = fin.tile([128, NPAIR * N], FD)
            nc.sync.dma_start(out=WI, in_=wi_in)

            def colblk(t, c0, n):
                return bass.AP(tensor=t.tensor, offset=t.offset + c0,
                               ap=[t.ap[0], [1, n]])

            def bcast_ak(t, c0):     # [TL, 64] col-block -> (a,k) bcast view
                return bass.AP(tensor=t.tensor, offset=t.offset + c0,
                               ap=[t.ap[0], [1, N], [0, K]])

            pi2 = prep.tile([TL, 1], FD)
            nc.vector.memset(pi2, float(np.pi / 2))
            zb = prep.tile([TL, 1], FD)
            nc.vector.memset(zb, 0.0)

            cosT = prep.tile([TL, N], FD)
            sinT = prep.tile([TL, N], FD)
            yaw_ap = colblk(YT, 2 * N, N)
            nc.scalar.activation(out=cosT, in_=yaw_ap, func=AF.Sin, bias=pi2)
            nc.scalar.activation(out=sinT, in_=yaw_ap, func=AF.Sin, bias=zb)

            # ---------- world disk coords [TL, (a,k)] ----------
            wx = prep.tile([TL, AK], FD)
            wy = prep.tile([TL, AK], FD)
            tmp = prep.tile([TL, AK], FD)
            tmq = prep.tile([TL, AK], FD)
            nc.vector.tensor_tensor(out=tmp, in0=CK, in1=bcast_ak(cosT, 0),
                                    op=AL.mult)
            nc.vector.tensor_tensor(out=wx, in0=tmp, in1=bcast_ak(YT, 0),
                                    op=AL.add)
            nc.vector.tensor_tensor(out=tmq, in0=CK, in1=bcast_ak(sinT, 0),
                                    op=AL.mult)
            nc.vector.tensor_tensor(out=wy, in0=bcast_ak(YT, N), in1=tmq,
                                    op=AL.subtract)

            # ---------- bf16 2-way coord split ----------
            def split2(src, name):
                a = prep.tile([TL, AK], BF, tag=name + "a")
                nc.vector.tensor_copy(a, src)
                r = prep.tile([TL, AK], FD, tag=name + "r")
                nc.gpsimd.tensor_tensor(out=r, in0=src, in1=a,
                                        op=AL.subtract)
                b = prep.tile([TL, AK], BF, tag=name + "b")
                nc.vector.tensor_copy(b, r)
                trunc = prep.tile([TL, AK], FD, tag=name + "t")
                nc.gpsimd.tensor_tensor(out=trunc, in0=src, in1=r,
                                        op=AL.subtract)
                nc.vector.tensor_tensor(out=trunc, in0=trunc, in1=b,
                                        op=AL.add)
                return a, b, trunc

            xa, xb, xtr = split2(wx, "x")
            ya, yb, ytr = split2(wy, "y")

            # sq from truncated coords, 3-way bf16 split
            s1 = prep.tile([TL, AK], FD)
            s2 = prep.tile([TL, AK], FD)
            nc.scalar.activation(out=s1, in_=xtr, func=AF.Square, bias=zb)
            nc.scalar.activation(out=s2, in_=ytr, func=AF.Square, bias=zb)
            sq = prep.tile([TL, AK], FD)
            nc.gpsimd.tensor_tensor(out=sq, in0=s1, in1=s2, op=AL.add)

            sa = prep.tile([TL, AK], BF)
            nc.vector.tensor_copy(sa, sq)
            r2 = prep.tile([TL, AK], FD)
            nc.gpsimd.tensor_tensor(out=r2, in0=sq, in1=sa, op=AL.subtract)
            sb = prep.tile([TL, AK], BF)
            nc.vector.tensor_copy(sb, r2)
            r3 = prep.tile([TL, AK], FD)
            nc.gpsimd.tensor_tensor(out=r3, in0=r2, in1=sb, op=AL.subtract)
            sc = prep.tile([TL, AK], BF)
            nc.vector.tensor_copy(sc, r3)

            def scale_m2(src, name):
                d = prep.tile([TL, AK], BF, tag=name)
                nc.vector.tensor_scalar(out=d, in0=src, scalar1=-2.0,
                                        scalar2=0.0, op0=AL.mult, op1=AL.add)
                return d

            m2xa = scale_m2(xa, "m2xa")
            m2xb = scale_m2(xb, "m2xb")
            m2ya = scale_m2(ya, "m2ya")
            m2yb = scale_m2(yb, "m2yb")

            onesA = prep.tile([TL, AK], BF)
            nc.vector.memset(onesA, 1.0)

            # ---------- (l,a) reorder for stationary side ----------
            def lreorder(src, name):
                d = prep.tile([TL, AK], BF, tag=name)
                nc.vector.tensor_copy(
                    bass.AP(tensor=d.tensor, offset=d.offset,
                            ap=[d.ap[0], [N, K], [1, N]]),
                    bass.AP(tensor=src.tensor, offset=src.offset,
                            ap=[src.ap[0], [1, K], [K, N]]))
                return d

            saL = lreorder(sa, "saL")
            sbL = lreorder(sb, "sbL")
            scL = lreorder(sc, "scL")
            xaL = lreorder(xa, "xaL")
            xbL = lreorder(xb, "xbL")
            yaL = lreorder(ya, "yaL")
            ybL = lreorder(yb, "ybL")

            # ---------- operand assembly (direct SBUF->SBUF) ----------
            S_ROWS = [saL, sbL, scL, onesA, onesA, onesA,
                      xaL, xaL, xbL, xbL, yaL, yaL, ybL, ybL]
            M_ROWS = [onesA, onesA, onesA, sa, sb, sc,
                      m2xa, m2xb, m2xa, m2xb, m2ya, m2yb, m2ya, m2yb]

            for h in range(2):
                for r in range(NR):
                    srcS = S_ROWS[r][h * NPAIR:(h + 1) * NPAIR, :]
                    row = SL[h * NR + r:h * NR + r + 1, :]
                    nc.scalar.dma_start(
                        out=bass.AP(tensor=SL.tensor,
                                    offset=row.offset + h * N,
                                    ap=[row.ap[0], [640, NPAIR],
                                        [128, 5], [1, N]]),
                        in_=bass.AP(tensor=srcS.tensor, offset=srcS.offset,
                                    ap=[srcS.ap[0], [N, 5], [1, N]]))
                    srcM = M_ROWS[r][h * NPAIR:(h + 1) * NPAIR, :]
                    vrow = V[h * NR + r:h * NR + r + 1, :]
                    nc.sync.dma_start(
                        out=bass.AP(tensor=V.tensor, offset=vrow.offset,
                                    ap=[vrow.ap[0], [AK, NPAIR], [1, AK]]),
                        in_=bass.AP(tensor=srcM.tensor, offset=srcM.offset,
                                    ap=[srcM.ap[0], [1, AK]]))

            # ---------- main loop + interleaved finish ----------
            dmin2 = fin.tile([128, NPAIR * N], BF)
            dist = fin.tile([128, NPAIR * N], FD)
            part = fin.tile([128, 2], FD)
            zb128 = fin.tile([128, 1], FD)
            nc.vector.memset(zb128, 0.0)

            def finish_chunk(ci):
                c0 = ci * 10 * N                      # 10 pairs * 64
                w = 10 * N
                dsl = bass.AP(tensor=dmin2.tensor, offset=dmin2.offset + c0,
                              ap=[dmin2.ap[0], [1, w]])
                nc.vector.tensor_scalar(out=dsl, in0=dsl, scalar1=0.0,
                                        scalar2=None, op0=AL.max)
                dstl = bass.AP(tensor=dist.tensor, offset=dist.offset + c0,
                               ap=[dist.ap[0], [1, w]])
                nc.scalar.activation(out=dstl, in_=dsl, func=AF.Sqrt,
                                     bias=zb128)
                t2 = mtmp.tile([128, w], FD, tag="t2")
                nc.vector.tensor_tensor(
                    out=t2, in0=dstl,
                    in1=bass.AP(tensor=WI.tensor, offset=WI.offset + c0,
                                ap=[WI.ap[0], [1, w]]),
                    op=AL.mult)
                u = mtmp.tile([128, w], FD, tag="u")
                nc.vector.tensor_tensor(
                    out=u,
                    in0=bass.AP(tensor=WM.tensor, offset=WM.offset + c0,
                                ap=[WM.ap[0], [1, w]]),
                    in1=t2, op=AL.subtract)
                nc.scalar.activation(out=u, in_=u, func=AF.Relu, bias=zb128,
                                     accum_out=part[:, ci:ci + 1])

            for p in range(NPAIR):
                P3 = p3pool.tile([128, 3 * 512], FD, tag="P3")
                P2 = p2pool.tile([128, 2 * 512], FD, tag="P2")
                for l in range(5):
                    dst = P3 if l < 3 else P2
                    c0 = 512 * l if l < 3 else 512 * (l - 3)
                    nc.tensor.matmul(
                        out=dst[0:128, c0:c0 + AK],
                        lhsT=SL[0:2 * NR, 640 * p + 128 * l:
                                640 * p + 128 * l + 128],
                        rhs=V[0:2 * NR, AK * p:AK * (p + 1)],
                        tile_position=(0, 0))

                dslice = bass.AP(tensor=dmin2.tensor,
                                 offset=dmin2.offset + p * N,
                                 ap=[dmin2.ap[0], [1, N]])
                if p in DIRECT:
                    m3 = mtmp.tile([128, N], FD, tag="m3")
                    m2t = mtmp.tile([128, N], FD, tag="m2t")
                    nc.vector.tensor_reduce(
                        out=m3,
                        in_=bass.AP(tensor=P3.tensor, offset=P3.offset,
                                    ap=[P3.ap[0], [K, N], [512, 3], [1, K]]),
                        axis=mybir.AxisListType.XY, op=AL.min)
                    nc.vector.tensor_reduce(
                        out=m2t,
                        in_=bass.AP(tensor=P2.tensor, offset=P2.offset,
                                    ap=[P2.ap[0], [K, N], [512, 2], [1, K]]),
                        axis=mybir.AxisListType.XY, op=AL.min)
                    nc.vector.tensor_tensor(out=dslice, in0=m3, in1=m2t,
                                            op=AL.min)
                else:
                    # flat bf16 D: cols l*320 + i*5 + k
                    Dt = dtile.tile([128, 5 * AK], BF, tag="D")
                    nc.scalar.activation(
                        out=bass.AP(tensor=Dt.tensor, offset=Dt.offset,
                                    ap=[Dt.ap[0], [AK, 3], [1, AK]]),
                        in_=bass.AP(tensor=P3.tensor, offset=P3.offset,
                                    ap=[P3.ap[0], [512, 3], [1, AK]]),
                        func=AF.Copy)
                    nc.scalar.activation(
                        out=bass.AP(tensor=Dt.tensor,
                                    offset=Dt.offset + 3 * AK,
                                    ap=[Dt.ap[0], [AK, 2], [1, AK]]),
                        in_=bass.AP(tensor=P2.tensor, offset=P2.offset,
                                    ap=[P2.ap[0], [512, 2], [1, AK]]),
                        func=AF.Copy)
                    if p in GPSD:
                        g1 = gtp.tile([128, AK], BF, tag="g1")
                        g2 = gtp.tile([128, AK], BF, tag="g2")
                        nc.gpsimd.tensor_tensor(
                            out=g1, in0=Dt[:, 0:AK], in1=Dt[:, AK:2 * AK],
                            op=AL.min)
                        nc.gpsimd.tensor_tensor(
                            out=g2, in0=Dt[:, 2 * AK:3 * AK],
                            in1=Dt[:, 3 * AK:4 * AK], op=AL.min)
                        nc.gpsimd.tensor_tensor(out=g1, in0=g1, in1=g2,
                                                op=AL.min)
                        nc.gpsimd.tensor_tensor(
                            out=g1, in0=g1, in1=Dt[:, 4 * AK:5 * AK],
                            op=AL.min)
                        nc.vector.tensor_reduce(
                            out=dslice,
                            in_=bass.AP(tensor=g1.tensor, offset=g1.offset,
                                        ap=[g1.ap[0], [K, N], [1, K]]),
                            axis=mybir.AxisListType.X, op=AL.min)
                    else:
                        nc.vector.tensor_reduce(
                            out=dslice,
                            in_=bass.AP(tensor=Dt.tensor, offset=Dt.offset,
                                        ap=[Dt.ap[0], [K, N], [AK, 5],
                                            [1, K]]),
                            axis=mybir.AxisListType.XY, op=AL.min)

                if p == 9:
                    finish_chunk(0)
                elif p == 19:
                    finish_chunk(1)

            nc.sync.dma_start(out=part_out, in_=part)

    nc.compile()
    return nc


def kernel(Y, length, width):
    Y = np.asarray(Y, np.float32)
    length = np.asarray(length, np.float32)
    width = np.asarray(width, np.float32)

    if "nc" not in _CACHE:
        _CACHE["nc"] = _build()
    nc = _CACHE["nc"]

    f2 = (2.0 * np.arange(K, dtype=np.float32) / (K - 1) - 1.0)
    ew = DECAY_RATE ** np.arange(T, dtype=np.float32)
    ew = (ew / ew.sum()).astype(np.float64)

    # prep-row rr = h*20 + p  <->  local slab t_local = 2p + h
    rr = np.arange(TL)
    tl_of_rr = 2 * (rr % NPAIR) + rr // NPAIR

    in_maps = []
    for c in range(NCORES):
        b, th = divmod(c, 2)
        t0 = th * TL
        tglob = t0 + tl_of_rr                       # [TL] global t per row

        yt = np.empty((TL, 3 * N), np.float32)
        yt[:, 0:N] = Y[b, :, tglob, 0]              # x[t, a]
        yt[:, N:2 * N] = Y[b, :, tglob, 1]          # y
        yt[:, 2 * N:3 * N] = Y[b, :, tglob, 4]      # yaw

        rad = width[b] / 2.0
        cmax = length[b] / 2.0 - rad                # [N]
        ck = (cmax[:, None] * f2[None, :]).reshape(AK).astype(np.float32)

        pd = rad[:, None] + rad[None, :] + BUFFER_DIST   # [j, i]
        ip = np.concatenate([1.0 / pd, 1.0 / pd], axis=0)  # [128, 64]

        wm = np.zeros((128, NPAIR * N), np.float64)
        mask = (~np.eye(N, dtype=bool)).astype(np.float64)   # [j, i]
        for p in range(NPAIR):
            for h in range(2):
                t = t0 + 2 * p + h
                wm[h * N:(h + 1) * N, p * N:(p + 1) * N] = \
                    mask * (ew[t] / (B * N * T))
        wi = wm * np.tile(ip, (1, NPAIR)).astype(np.float64)

        in_maps.append({
            "yt_in": yt, "ck_in": ck,
            "wm_in": wm.astype(np.float32),
            "wi_in": wi.astype(np.float32),
            "z_in": np.zeros(SLW, ml_dtypes.bfloat16),
        })

    global _LAST_INMAPS
    _LAST_INMAPS = in_maps
    res = bass_utils.run_bass_kernel_spmd(nc, in_maps,
                                          core_ids=list(range(NCORES)))
    total = 0.0
    for c in range(NCORES):
        total += float(res.results[c]["part_out"].astype(np.float64).sum())
    return np.float32(total)
